# revision 26
# baseline (speedup 1.0000x reference)
"""DeepSeek MLA prefill (absorbed) on 8 Trainium2 NeuronCores — v3.

Sharding: tensor-parallel over heads (2 of 16 per core). The QKV-compression
and Q-uncompression GEMMs are fused on the host (W_qeff = W_cqkv_q @ W_uq per
head block), so each core computes its own heads' q directly from the full x
with NO q collective. Attention is the dense count-matrix formulation
(softmax over all 4096 kv positions weighted by top-k multiplicity), run
head-sequentially; all kv/count/value tiles are loaded once during head 0
and stay resident in SBUF for head 1, so head 0's o2 AllGather flies over an
idle DMA fabric. Head 1's AllGather hides under the head-0 half of the
O-projection accumulation.

All PE operands are f16 (full rate, half the SBUF/DMA energy of f32r — the
board power-throttles under sustained load); accumulation stays in f32 PSUM.
Inputs are host-packed so every DMA moves >=0.5 MB with partition-dim 128.
"""

import os
import sys

sys.path.insert(0, "/opt/trn_rl_repo")

import numpy as np

import concourse.bass as bass
import concourse.tile as tile
from concourse import bacc, mybir
from concourse.bass_utils import run_bass_kernel_spmd

F32 = mybir.dt.float32
F16 = mybir.dt.float16
NP16 = np.float16

N_CORES = 8
M = 512
HID = 7168
H_LOC = 2
S_KV = 4096
D_KV_C = 512
OUT_C = HID // N_CORES          # 896
SM_SCALE = 1.0 / float(np.sqrt(np.float32(576)))

KH = HID // 128                 # 56 contraction chunks for fused q GEMM
KG = 8                          # chunks per phase-A DMA group
NSC = S_KV // 128               # 32 kv chunks
NQ = NSC // 4                   # 8 stream groups of 4 kv chunks
N_WARM = 20


def build_program():
    nc = bacc.Bacc("TRN2", target_bir_lowering=False, debug=False,
                   num_devices=N_CORES)

    # all packed [128, ...] host-side; see prep_inputs for layouts
    xT = nc.dram_tensor("xT", [128, KH * M], F16, kind="ExternalInput")
    wqe = nc.dram_tensor("wqe", [128, KH * 384], F16, kind="ExternalInput")
    shf = nc.dram_tensor("shf", [128, 64], F16, kind="ExternalInput")
    wqk = nc.dram_tensor("wqk", [H_LOC, 128, 512], F16, kind="ExternalInput")
    kvA = nc.dram_tensor("kvA", [128, NSC * 512], F16, kind="ExternalInput")
    kvB = nc.dram_tensor("kvB", [64, S_KV], F16, kind="ExternalInput")
    vv = nc.dram_tensor("vv", [128, NSC * 512], F16, kind="ExternalInput")
    cnt = nc.dram_tensor("cnt", [128, NSC * 512], F16, kind="ExternalInput")
    wo1 = nc.dram_tensor("wo1", [H_LOC, 512, 128], F16, kind="ExternalInput")
    wop = nc.dram_tensor("wop", [128, 16 * OUT_C], F16, kind="ExternalInput")
    outT = nc.dram_tensor("outT", [OUT_C, M], F16, kind="ExternalOutput")

    rg = [list(range(N_CORES))]

    with tile.TileContext(nc) as tc, \
            nc.allow_low_precision(reason="f16 matmul pipeline"):
        with tc.tile_pool(name="dram", bufs=1, space="DRAM") as dram:
            o2_loc = [[dram.tile([128, M], F16, name=f"o2loc{h}")]
                      for h in range(H_LOC)]
            o2_all = [[dram.tile([128 * N_CORES, M], F16, name=f"o2all{h}",
                                 addr_space="Shared")]
                      for h in range(H_LOC)]
            ccw_in = dram.tile([1, 16], F16, name="ccw_in")
            ccw_out = dram.tile([N_CORES, 16], F16, name="ccw_out",
                                addr_space="Shared")

            glob_cm = tc.tile_pool(name="glob", bufs=1)
            glob = glob_cm.__enter__()

            ones_col_f = glob.tile([128, 1], F32, name="ones_col_f")
            nc.vector.memset(ones_col_f[:], 1.0)
            ones_col = glob.tile([128, 1], F16, name="ones_col")
            nc.vector.tensor_copy(ones_col[:], ones_col_f[:])
            ones_row_f = glob.tile([1, 128], F32, name="ones_row_f")
            nc.vector.memset(ones_row_f[:], 1.0)
            ones_row = glob.tile([1, 128], F16, name="ones_row")
            nc.vector.tensor_copy(ones_row[:], ones_row_f[:])

            shft = glob.tile([128, 64], F16, name="shft")
            nc.sync.dma_start(shft[:], shf[:, :])

            nope = [glob.tile([128, M], F16, name=f"nope{h}")
                    for h in range(H_LOC)]
            peboth = glob.tile([128, M], F16, name="peboth")
            pe1 = glob.tile([64, M], F16, name="pe1")

            # ---------------- phase A: q = x @ wqe (fused s1+s2) ----------
            with (
                tc.tile_pool(name="sA", bufs=3) as sA,
                tc.tile_pool(name="psA", bufs=1, space="PSUM") as psA,
                tc.tile_pool(name="psW", bufs=1, space="PSUM") as psW,
            ):
                warm = glob.tile([128, 64], F32, name="warm")
                nc.vector.memset(warm[:], 0.0)
                wps = psW.tile([1, 64], F32, name="wps")
                for i in range(N_WARM):
                    nc.tensor.matmul(wps[:], warm[:, 0:1], warm[:],
                                     start=(i == 0), stop=(i == N_WARM - 1),
                                     skip_group_check=True)
                accA = [psA.tile([128, M], F32, name=f"accA{g}")
                        for g in range(3)]
                # dummy collective: pays the CC init latency during phase A
                # and resyncs startup skew across the 8 cores
                ccw_sb = glob.tile([1, 16], F16, name="ccw_sb")
                nc.vector.memset(ccw_sb[:], 0.0)
                nc.sync.dma_start(ccw_in[:], ccw_sb[:])
                nc.gpsimd.collective_compute(
                    "AllGather", mybir.AluOpType.bypass, replica_groups=rg,
                    ins=[ccw_in.opt()], outs=[ccw_out.opt()])
                groups = ([(0, 2), (2, 2), (4, 2), (6, 2), (8, 4), (12, 4)]
                          + [(16, 8), (24, 8), (32, 8), (40, 8), (48, 8)])
                for k0, gsz in groups:
                    xg = sA.tile([128, gsz * M], F16, name="xg", tag="xg")
                    nc.sync.dma_start(
                        xg[:], xT[:, k0 * M:(k0 + gsz) * M])
                    wg = sA.tile([128, gsz * 384], F16, name="wg", tag="wg")
                    # second DMA queue so phase A is not issue-rate bound
                    nc.gpsimd.dma_start(
                        wg[:], wqe[:, k0 * 384:(k0 + gsz) * 384])
                    for j in range(gsz):
                        k = k0 + j
                        for g in range(3):
                            nc.tensor.matmul(
                                accA[g][:],
                                wg[:, j * 384 + g * 128:
                                   j * 384 + (g + 1) * 128],
                                xg[:, j * M:(j + 1) * M],
                                start=(k == 0), stop=(k == KH - 1),
                                skip_group_check=True)
                # small weights after the first phase-A groups in the queue
                wqkt = []
                for h in range(H_LOC):
                    wh = glob.tile([128, 512], F16, name=f"wqk{h}")
                    nc.sync.dma_start(wh[:], wqk[h])
                    wqkt.append(wh)
                wo1t = [[None] * 4 for _ in range(H_LOC)]
                for h in range(H_LOC):
                    for kc in range(4):
                        wk = glob.tile([128, 128], F16, name=f"wo1_{h}_{kc}")
                        nc.sync.dma_start(
                            wk[:], wo1[h][kc * 128:(kc + 1) * 128, :])
                        wo1t[h][kc] = wk
                nc.vector.tensor_copy(nope[0][:], accA[0][:])
                nc.vector.tensor_copy(peboth[:], accA[1][:])
                nc.vector.tensor_copy(nope[1][:], accA[2][:])
                # h1 pe rows live at partitions 64..127; shift to 0..63
                pshf = psW.tile([64, M], F32, name="pshf")
                nc.tensor.matmul(pshf[:], shft[:], peboth[:],
                                 start=True, stop=True)
                nc.vector.tensor_copy(pe1[:], pshf[:])
            pe = [peboth[0:64, :], pe1[:]]

            # stage 3 for both heads up front (q_abs = W_qk^T-slices @ nope)
            qf_all = [[None] * 5 for _ in range(H_LOC)]
            with tc.tile_pool(name="ps3", bufs=2, space="PSUM") as ps3:
                for h in range(H_LOC):
                    for c in range(4):
                        acc = ps3.tile([128, M], F32, name="acc3",
                                       tag="acc3")
                        nc.tensor.matmul(
                            acc[:], wqkt[h][:, c * 128:(c + 1) * 128],
                            nope[h][:], start=True, stop=True)
                        qb = glob.tile([128, M], F16, name=f"qf{h}_{c}")
                        nc.vector.tensor_copy(qb[:], acc[:])
                        qf_all[h][c] = qb
                    qf_all[h][4] = pe[h]

            # resident attention streams (loaded once, reused by head 1)
            kva4 = [glob.tile([128, 2048], F16, name=f"kva4_{q}")
                    for q in range(NQ)]
            kvb4 = [glob.tile([64, 512], F16, name=f"kvb4_{q}")
                    for q in range(NQ)]
            cc4 = [glob.tile([128, 2048], F16, name=f"cc4_{q}")
                   for q in range(NQ)]
            vt4 = [glob.tile([128, 2048], F16, name=f"vt4_{q}")
                   for q in range(NQ)]
            wopt = [glob.tile([128, 4 * OUT_C], F16, name=f"wop4_{j}")
                    for j in range(4)]
            o2t = [[None] * 8 for _ in range(H_LOC)]

            zb_sb = [glob.tile([128, M], F16, name=f"zs{h}")
                     for h in range(H_LOC)]

            for h in range(H_LOC):
                att_cm = tc.tile_pool(name=f"att{h}", bufs=1)
                att = att_cm.__enter__()
                if h == 1:
                    # o2_all[0] reads: emitted BEFORE the later AllGathers
                    # so they wait only on the first collective
                    for k in range(8):
                        ok = glob.tile([128, M], F16, name=f"o2a0_{k}")
                        nc.sync.dma_start(
                            ok[:], o2_all[0][0][k * 128:(k + 1) * 128, :])
                        o2t[0][k] = ok

                qf = qf_all[h]
                pt = att.tile([128, NSC * M], F16, name=f"pt{h}")
                z_sb = att.tile([1, M], F32, name=f"z{h}")
                rz = att.tile([1, M], F16, name=f"rz{h}")
                zacc = att.tile([128, M], F32, name=f"zacc{h}")
                zf16 = att.tile([128, M], F16, name=f"zf16_{h}")

                with (
                    tc.tile_pool(name=f"exps{h}", bufs=4) as exps,
                    tc.tile_pool(name=f"psS{h}", bufs=3, space="PSUM") as psS,
                ):
                    for sc in range(NSC):
                        q, r = divmod(sc, 4)
                        if h == 0 and r == 0:
                            nc.sync.dma_start(
                                kva4[q][:],
                                kvA[:, q * 2048:(q + 1) * 2048])
                            nc.sync.dma_start(
                                kvb4[q][:], kvB[:, q * 512:(q + 1) * 512])
                            nc.sync.dma_start(
                                cc4[q][:], cnt[:, q * 2048:(q + 1) * 2048])
                        if h == 0 and sc == 28:
                            for q2 in range(2):  # early value-tile prefetch
                                nc.sync.dma_start(
                                    vt4[q2][:],
                                    vv[:, q2 * 2048:(q2 + 1) * 2048])
                        acc = psS.tile([128, M], F32, name="accS", tag="accS")
                        for j in range(4):
                            nc.tensor.matmul(
                                acc[:],
                                kva4[q][:, r * 512 + j * 128:
                                        r * 512 + (j + 1) * 128],
                                qf[j][:], start=(j == 0), stop=False)
                        nc.tensor.matmul(
                            acc[:], kvb4[q][:, r * 128:(r + 1) * 128],
                            qf[4], start=False, stop=True)
                        ex = exps.tile([128, M], F16, name="ex", tag="ex")
                        nc.scalar.activation(
                            ex[:], acc[:], mybir.ActivationFunctionType.Exp,
                            scale=SM_SCALE)
                        psl = pt[:, sc * M:(sc + 1) * M]
                        nc.vector.tensor_mul(
                            psl, ex[:], cc4[q][:, r * M:(r + 1) * M])
                        # running Z on the DVE, right behind the multiply
                        if sc == 0:
                            nc.vector.tensor_copy(zacc[:], psl)
                        else:
                            nc.vector.tensor_add(zacc[:], zacc[:], psl)
                    nc.vector.tensor_copy(zf16[:], zacc[:])

                if h == 0:
                    for q in range(2, NQ):
                        nc.sync.dma_start(
                            vt4[q][:], vv[:, q * 2048:(q + 1) * 2048])
                    for j in range(4):
                        nc.sync.dma_start(
                            wopt[j][:],
                            wop[:, j * 4 * OUT_C:(j + 1) * 4 * OUT_C])

                # value phase (head 1 in token-halves so its first o2
                # AllGather launches at the midpoint)
                o_sb = [att.tile([128, M], F16, name=f"o_{h}_{c}")
                        for c in range(4)]
                o2s = att.tile([128, M], F16, name=f"o2s{h}")
                halves = [(0, M)]
                with (
                    tc.tile_pool(name=f"psO{h}", bufs=1, space="PSUM") as psO,
                    tc.tile_pool(name=f"psB{h}", bufs=1, space="PSUM") as psB,
                ):
                    zsum = psB.tile([1, M], F32, name="zsum")
                    zb = psB.tile([128, M], F32, name="zb")
                    acc5h = [psB.tile([128, hi - lo], F32, name=f"acc5_{hv}")
                             for hv, (lo, hi) in enumerate(halves)]
                    o_ps = [psO.tile([128, M], F32, name=f"op{c}")
                            for c in range(4)]
                    for hv, (lo, hi) in enumerate(halves):
                        w = hi - lo
                        for sc in range(NSC):
                            q, r = divmod(sc, 4)
                            psl = pt[:, sc * M + lo:sc * M + hi]
                            for c in range(4):
                                nc.tensor.matmul(
                                    o_ps[c][:, 0:w],
                                    vt4[q][:, r * 512 + c * 128:
                                           r * 512 + (c + 1) * 128],
                                    psl, start=(sc == 0),
                                    stop=(sc == NSC - 1),
                                    skip_group_check=True)
                            if hv == 0 and sc == 2:
                                nc.tensor.matmul(zsum[:], ones_col[:],
                                                 zf16[:], start=True,
                                                 stop=True,
                                                 skip_group_check=True)
                                nc.vector.tensor_copy(z_sb[:], zsum[:])
                                nc.vector.reciprocal(rz[:], z_sb[:])
                            if hv == 0 and sc == 8:
                                nc.tensor.matmul(zb[:], ones_row[:], rz[:],
                                                 start=True, stop=True,
                                                 skip_group_check=True)
                                nc.vector.tensor_copy(zb_sb[h][:], zb[:])
                        for c in range(4):
                            nc.vector.tensor_copy(o_sb[c][:, lo:hi],
                                                  o_ps[c][:, 0:w])
                        for kc in range(4):
                            nc.tensor.matmul(
                                acc5h[hv][:], wo1t[h][kc][:],
                                o_sb[kc][:, lo:hi],
                                start=(kc == 0), stop=(kc == 3),
                                skip_group_check=True)
                        nc.vector.tensor_mul(o2s[:, lo:hi], acc5h[hv][:],
                                             zb_sb[h][:, lo:hi])
                        nc.sync.dma_start(o2_loc[h][hv][:], o2s[:, lo:hi])
                        nc.gpsimd.collective_compute(
                            "AllGather", mybir.AluOpType.bypass,
                            replica_groups=rg,
                            ins=[o2_loc[h][hv].opt()],
                            outs=[o2_all[h][hv].opt()])
                att_cm.__exit__(None, None, None)

            # ---------------- O-projection --------------------------------
            def wslice(j, p):
                # stationary [128,128] for contraction block j, out chunk p
                return wopt[j // 4][:, (j % 4) * OUT_C + p * 128:
                                   (j % 4) * OUT_C + (p + 1) * 128]

            with (
                tc.tile_pool(name="s6", bufs=1) as s6,
                tc.tile_pool(name="ps6", bufs=1, space="PSUM") as ps6,
                tc.tile_pool(name="s6o", bufs=3) as s6o,
            ):
                acc6 = [ps6.tile([128, M], F32, name=f"acc6_{p}")
                        for p in range(7)]
                # part A: head-0 blocks only (hides head-1's AllGather)
                for k in range(8):
                    for p in range(7):
                        nc.tensor.matmul(
                            acc6[p][:], wslice(k, p), o2t[0][k][:],
                            start=(k == 0), stop=False,
                            skip_group_check=True)
                # part B: head-1 blocks (k-major so the first read
                # unblocks compute while the rest stream in)
                o2t1 = [None] * 8
                for k in range(8):
                    ok = s6.tile([128, M], F16, name=f"o2a1_{k}")
                    nc.sync.dma_start(
                        ok[:], o2_all[1][0][k * 128:(k + 1) * 128, :])
                    o2t1[k] = ok
                for k in range(7):
                    for p in range(7):
                        nc.tensor.matmul(
                            acc6[p][:], wslice(8 + k, p), o2t1[k][:],
                            start=False, stop=False,
                            skip_group_check=True)
                for p in range(7):
                    nc.tensor.matmul(
                        acc6[p][:], wslice(15, p), o2t1[7][:],
                        start=False, stop=True, skip_group_check=True)
                    ob = s6o.tile([128, M], F16, name="outb", tag="outb")
                    if p % 2 == 0:
                        nc.vector.tensor_copy(ob[:], acc6[p][:])
                    else:
                        nc.scalar.activation(
                            ob[:], acc6[p][:],
                            mybir.ActivationFunctionType.Copy)
                    nc.sync.dma_start(outT[p * 128:(p + 1) * 128, :], ob[:])

            glob_cm.__exit__(None, None, None)

    nc.compile()
    return nc


def _pack(a, nblk):
    """[nblk*128, W] -> [128, nblk*W] with block-major free layout."""
    w = a.shape[1]
    return np.ascontiguousarray(
        a.reshape(nblk, 128, w).transpose(1, 0, 2).reshape(128, nblk * w))


def prep_inputs(x, W_cqkv, W_uq, W_qk, kv_cache, W_o1, W_oproj, indices):
    x = np.asarray(x, np.float32)
    W_cqkv = np.asarray(W_cqkv, np.float32)
    W_uq = np.asarray(W_uq, np.float32)
    W_qk = np.asarray(W_qk, np.float32)
    kv_cache = np.asarray(kv_cache, np.float32)
    W_o1 = np.asarray(W_o1, np.float32)
    W_oproj = np.asarray(W_oproj, np.float32)
    indices = np.asarray(indices)

    xTp = _pack(np.ascontiguousarray(x.T), KH).astype(NP16)
    wq_full = W_cqkv[:, D_KV_C:D_KV_C + 1536]

    kvT = np.ascontiguousarray(kv_cache.T)  # [576, 4096]
    # kvA[p, sc*512 + j*128 + c] = kv dim (j*128+p) at position (sc*128+c)
    kvAf = np.ascontiguousarray(
        kvT[:512].reshape(4, 128, NSC, 128).transpose(1, 2, 0, 3)
        .reshape(128, NSC * 512)).astype(NP16)
    kvBf = np.ascontiguousarray(kvT[512:]).astype(NP16)
    vvP = _pack(np.ascontiguousarray(kv_cache[:, :D_KV_C]),
                NSC).astype(NP16)

    cm = np.bincount(
        (np.arange(M, dtype=np.int64)[:, None] * S_KV + indices).ravel(),
        minlength=M * S_KV).reshape(M, S_KV)
    cntP = _pack(np.ascontiguousarray(cm.T.astype(np.float32)),
                 NSC).astype(NP16)

    shf_np = np.zeros((128, 64), NP16)
    shf_np[np.arange(64) + 64, np.arange(64)] = 1.0

    in_maps = []
    for i in range(N_CORES):
        h0 = i * H_LOC
        c0 = i * OUT_C
        # fused q weights; cols [h0 nope | h0 pe ; h1 pe | h1 nope]
        wu = W_uq[:, h0 * 192:(h0 + H_LOC) * 192]
        cols = np.concatenate([
            wu[:, 0:128], wu[:, 128:192], wu[:, 320:384], wu[:, 192:320],
        ], axis=1)
        wqe_i = _pack((wq_full @ cols), KH).astype(NP16)
        # O-proj rows: head-0 rank blocks then head-1 rank blocks
        wop_rows = []
        for h in range(H_LOC):
            for k in range(N_CORES):
                g = k * H_LOC + h
                wop_rows.append(W_oproj[g * 128:(g + 1) * 128,
                                        c0:c0 + OUT_C])
        wop_i = _pack(np.concatenate(wop_rows, 0), 16).astype(NP16)
        in_maps.append({
            "xT": xTp,
            "wqe": wqe_i,
            "shf": shf_np,
            "wqk": W_qk[h0:h0 + H_LOC].astype(NP16),
            "kvA": kvAf,
            "kvB": kvBf,
            "vv": vvP,
            "cnt": cntP,
            "wo1": W_o1[h0:h0 + H_LOC].astype(NP16),
            "wop": wop_i,
        })
    return in_maps


_prog_cache = {}


def kernel(x, W_cqkv, W_uq, W_qk, kv_cache, W_o1, W_oproj, indices):
    if "nc" not in _prog_cache:
        _prog_cache["nc"] = build_program()
    nc = _prog_cache["nc"]
    in_maps = prep_inputs(x, W_cqkv, W_uq, W_qk, kv_cache, W_o1, W_oproj,
                          indices)
    trace = bool(int(os.environ.get("KERNEL_TRACE", "0")))
    res = run_bass_kernel_spmd(nc, in_maps, list(range(N_CORES)),
                               trace=trace)
    _prog_cache["last_result"] = res
    out = np.empty((M, HID), np.float32)
    for i in range(N_CORES):
        out[:, i * OUT_C:(i + 1) * OUT_C] = res.results[i]["outT"].T
    return out


# revision 27
# speedup vs baseline: 1.0143x; 1.0143x over previous
"""DeepSeek MLA prefill (absorbed) on 8 Trainium2 NeuronCores — v3.

Sharding: tensor-parallel over heads (2 of 16 per core). The QKV-compression
and Q-uncompression GEMMs are fused on the host (W_qeff = W_cqkv_q @ W_uq per
head block), so each core computes its own heads' q directly from the full x
with NO q collective. Attention is the dense count-matrix formulation
(softmax over all 4096 kv positions weighted by top-k multiplicity), run
head-sequentially; all kv/count/value tiles are loaded once during head 0
and stay resident in SBUF for head 1, so head 0's o2 AllGather flies over an
idle DMA fabric. Head 1's AllGather hides under the head-0 half of the
O-projection accumulation.

All PE operands are f16 (full rate, half the SBUF/DMA energy of f32r — the
board power-throttles under sustained load); accumulation stays in f32 PSUM.
Inputs are host-packed so every DMA moves >=0.5 MB with partition-dim 128.
"""

import os
import sys

sys.path.insert(0, "/opt/trn_rl_repo")

import numpy as np

import concourse.bass as bass
import concourse.tile as tile
from concourse import bacc, mybir
from concourse.bass_utils import run_bass_kernel_spmd

F32 = mybir.dt.float32
F16 = mybir.dt.float16
NP16 = np.float16

N_CORES = 8
M = 512
HID = 7168
H_LOC = 2
S_KV = 4096
D_KV_C = 512
OUT_C = HID // N_CORES          # 896
SM_SCALE = 1.0 / float(np.sqrt(np.float32(576)))

KH = HID // 128                 # 56 contraction chunks for fused q GEMM
KG = 8                          # chunks per phase-A DMA group
NSC = S_KV // 128               # 32 kv chunks
NQ = NSC // 4                   # 8 stream groups of 4 kv chunks
N_WARM = 20


def build_program():
    nc = bacc.Bacc("TRN2", target_bir_lowering=False, debug=False,
                   num_devices=N_CORES)

    # all packed [128, ...] host-side; see prep_inputs for layouts
    xT = nc.dram_tensor("xT", [128, KH * M], F16, kind="ExternalInput")
    wqe = nc.dram_tensor("wqe", [128, KH * 384], F16, kind="ExternalInput")
    shf = nc.dram_tensor("shf", [128, 64], F16, kind="ExternalInput")
    wqk = nc.dram_tensor("wqk", [H_LOC, 128, 512], F16, kind="ExternalInput")
    kvA = nc.dram_tensor("kvA", [128, NSC * 512], F16, kind="ExternalInput")
    kvB = nc.dram_tensor("kvB", [64, S_KV], F16, kind="ExternalInput")
    vv = nc.dram_tensor("vv", [128, NSC * 512], F16, kind="ExternalInput")
    cnt = nc.dram_tensor("cnt", [128, NSC * 512], F16, kind="ExternalInput")
    wo1 = nc.dram_tensor("wo1", [H_LOC, 512, 128], F16, kind="ExternalInput")
    wop = nc.dram_tensor("wop", [128, 16 * OUT_C], F16, kind="ExternalInput")
    outT = nc.dram_tensor("outT", [OUT_C, M], F16, kind="ExternalOutput")

    rg = [list(range(N_CORES))]

    with tile.TileContext(nc) as tc, \
            nc.allow_low_precision(reason="f16 matmul pipeline"):
        with tc.tile_pool(name="dram", bufs=1, space="DRAM") as dram:
            o2_loc = [[dram.tile([128, M], F16, name=f"o2loc{h}")]
                      for h in range(H_LOC)]
            o2_all = [[dram.tile([128 * N_CORES, M], F16, name=f"o2all{h}",
                                 addr_space="Shared")]
                      for h in range(H_LOC)]
            ccw_in = dram.tile([1, 16], F16, name="ccw_in")
            ccw_out = dram.tile([N_CORES, 16], F16, name="ccw_out",
                                addr_space="Shared")

            glob_cm = tc.tile_pool(name="glob", bufs=1)
            glob = glob_cm.__enter__()

            ones_col_f = glob.tile([128, 1], F32, name="ones_col_f")
            nc.vector.memset(ones_col_f[:], 1.0)
            ones_col = glob.tile([128, 1], F16, name="ones_col")
            nc.vector.tensor_copy(ones_col[:], ones_col_f[:])
            ones_row_f = glob.tile([1, 128], F32, name="ones_row_f")
            nc.vector.memset(ones_row_f[:], 1.0)
            ones_row = glob.tile([1, 128], F16, name="ones_row")
            nc.vector.tensor_copy(ones_row[:], ones_row_f[:])

            shft = glob.tile([128, 64], F16, name="shft")
            nc.sync.dma_start(shft[:], shf[:, :])

            nope = [glob.tile([128, M], F16, name=f"nope{h}")
                    for h in range(H_LOC)]
            peboth = glob.tile([128, M], F16, name="peboth")
            pe1 = glob.tile([64, M], F16, name="pe1")

            # ---------------- phase A: q = x @ wqe (fused s1+s2) ----------
            with (
                tc.tile_pool(name="sA", bufs=3) as sA,
                tc.tile_pool(name="psA", bufs=1, space="PSUM") as psA,
                tc.tile_pool(name="psW", bufs=1, space="PSUM") as psW,
            ):
                warm = glob.tile([128, 64], F32, name="warm")
                nc.vector.memset(warm[:], 0.0)
                wps = psW.tile([1, 64], F32, name="wps")
                for i in range(N_WARM):
                    nc.tensor.matmul(wps[:], warm[:, 0:1], warm[:],
                                     start=(i == 0), stop=(i == N_WARM - 1),
                                     skip_group_check=True)
                accA = [psA.tile([128, M], F32, name=f"accA{g}")
                        for g in range(3)]
                # dummy collective: pays the CC init latency during phase A
                # and resyncs startup skew across the 8 cores
                ccw_sb = glob.tile([1, 16], F16, name="ccw_sb")
                nc.vector.memset(ccw_sb[:], 0.0)
                nc.sync.dma_start(ccw_in[:], ccw_sb[:])
                nc.gpsimd.collective_compute(
                    "AllGather", mybir.AluOpType.bypass, replica_groups=rg,
                    ins=[ccw_in.opt()], outs=[ccw_out.opt()])
                groups = ([(0, 2), (2, 2), (4, 2), (6, 2), (8, 4), (12, 4)]
                          + [(16, 8), (24, 8), (32, 8), (40, 8), (48, 8)])
                for k0, gsz in groups:
                    xg = sA.tile([128, gsz * M], F16, name="xg", tag="xg")
                    nc.sync.dma_start(
                        xg[:], xT[:, k0 * M:(k0 + gsz) * M])
                    wg = sA.tile([128, gsz * 384], F16, name="wg", tag="wg")
                    # second DMA queue so phase A is not issue-rate bound
                    # (NOT gpsimd: the warmup collective would block it)
                    nc.scalar.dma_start(
                        wg[:], wqe[:, k0 * 384:(k0 + gsz) * 384])
                    for j in range(gsz):
                        k = k0 + j
                        for g in range(3):
                            nc.tensor.matmul(
                                accA[g][:],
                                wg[:, j * 384 + g * 128:
                                   j * 384 + (g + 1) * 128],
                                xg[:, j * M:(j + 1) * M],
                                start=(k == 0), stop=(k == KH - 1),
                                skip_group_check=True)
                # small weights after the first phase-A groups in the queue
                wqkt = []
                for h in range(H_LOC):
                    wh = glob.tile([128, 512], F16, name=f"wqk{h}")
                    nc.sync.dma_start(wh[:], wqk[h])
                    wqkt.append(wh)
                wo1t = [[None] * 4 for _ in range(H_LOC)]
                for h in range(H_LOC):
                    for kc in range(4):
                        wk = glob.tile([128, 128], F16, name=f"wo1_{h}_{kc}")
                        nc.sync.dma_start(
                            wk[:], wo1[h][kc * 128:(kc + 1) * 128, :])
                        wo1t[h][kc] = wk
                nc.vector.tensor_copy(nope[0][:], accA[0][:])
                nc.vector.tensor_copy(peboth[:], accA[1][:])
                nc.vector.tensor_copy(nope[1][:], accA[2][:])
                # h1 pe rows live at partitions 64..127; shift to 0..63
                pshf = psW.tile([64, M], F32, name="pshf")
                nc.tensor.matmul(pshf[:], shft[:], peboth[:],
                                 start=True, stop=True)
                nc.vector.tensor_copy(pe1[:], pshf[:])
            pe = [peboth[0:64, :], pe1[:]]

            # stage 3 for both heads up front (q_abs = W_qk^T-slices @ nope)
            qf_all = [[None] * 5 for _ in range(H_LOC)]
            with tc.tile_pool(name="ps3", bufs=2, space="PSUM") as ps3:
                for h in range(H_LOC):
                    for c in range(4):
                        acc = ps3.tile([128, M], F32, name="acc3",
                                       tag="acc3")
                        nc.tensor.matmul(
                            acc[:], wqkt[h][:, c * 128:(c + 1) * 128],
                            nope[h][:], start=True, stop=True)
                        qb = glob.tile([128, M], F16, name=f"qf{h}_{c}")
                        nc.vector.tensor_copy(qb[:], acc[:])
                        qf_all[h][c] = qb
                    qf_all[h][4] = pe[h]

            # resident attention streams (loaded once, reused by head 1)
            kva4 = [glob.tile([128, 2048], F16, name=f"kva4_{q}")
                    for q in range(NQ)]
            kvb4 = [glob.tile([64, 512], F16, name=f"kvb4_{q}")
                    for q in range(NQ)]
            cc4 = [glob.tile([128, 2048], F16, name=f"cc4_{q}")
                   for q in range(NQ)]
            vt4 = [glob.tile([128, 2048], F16, name=f"vt4_{q}")
                   for q in range(NQ)]
            wopt = [glob.tile([128, 4 * OUT_C], F16, name=f"wop4_{j}")
                    for j in range(4)]
            o2t = [[None] * 8 for _ in range(H_LOC)]

            zb_sb = [glob.tile([128, M], F16, name=f"zs{h}")
                     for h in range(H_LOC)]

            for h in range(H_LOC):
                att_cm = tc.tile_pool(name=f"att{h}", bufs=1)
                att = att_cm.__enter__()
                if h == 1:
                    # o2_all[0] reads: emitted BEFORE the later AllGathers
                    # so they wait only on the first collective
                    for k in range(8):
                        ok = glob.tile([128, M], F16, name=f"o2a0_{k}")
                        nc.sync.dma_start(
                            ok[:], o2_all[0][0][k * 128:(k + 1) * 128, :])
                        o2t[0][k] = ok

                qf = qf_all[h]
                pt = att.tile([128, NSC * M], F16, name=f"pt{h}")
                z_sb = att.tile([1, M], F32, name=f"z{h}")
                rz = att.tile([1, M], F16, name=f"rz{h}")
                zacc = att.tile([128, M], F32, name=f"zacc{h}")
                zf16 = att.tile([128, M], F16, name=f"zf16_{h}")

                with (
                    tc.tile_pool(name=f"exps{h}", bufs=4) as exps,
                    tc.tile_pool(name=f"psS{h}", bufs=3, space="PSUM") as psS,
                ):
                    for sc in range(NSC):
                        q, r = divmod(sc, 4)
                        if h == 0 and r == 0:
                            nc.sync.dma_start(
                                kva4[q][:],
                                kvA[:, q * 2048:(q + 1) * 2048])
                            nc.sync.dma_start(
                                kvb4[q][:], kvB[:, q * 512:(q + 1) * 512])
                            nc.sync.dma_start(
                                cc4[q][:], cnt[:, q * 2048:(q + 1) * 2048])
                        if h == 0 and sc == 28:
                            for q2 in range(2):  # early value-tile prefetch
                                nc.sync.dma_start(
                                    vt4[q2][:],
                                    vv[:, q2 * 2048:(q2 + 1) * 2048])
                        acc = psS.tile([128, M], F32, name="accS", tag="accS")
                        for j in range(4):
                            nc.tensor.matmul(
                                acc[:],
                                kva4[q][:, r * 512 + j * 128:
                                        r * 512 + (j + 1) * 128],
                                qf[j][:], start=(j == 0), stop=False)
                        nc.tensor.matmul(
                            acc[:], kvb4[q][:, r * 128:(r + 1) * 128],
                            qf[4], start=False, stop=True)
                        ex = exps.tile([128, M], F16, name="ex", tag="ex")
                        nc.scalar.activation(
                            ex[:], acc[:], mybir.ActivationFunctionType.Exp,
                            scale=SM_SCALE)
                        psl = pt[:, sc * M:(sc + 1) * M]
                        nc.vector.tensor_mul(
                            psl, ex[:], cc4[q][:, r * M:(r + 1) * M])
                        # running Z on the DVE, right behind the multiply
                        if sc == 0:
                            nc.vector.tensor_copy(zacc[:], psl)
                        else:
                            nc.vector.tensor_add(zacc[:], zacc[:], psl)
                    nc.vector.tensor_copy(zf16[:], zacc[:])

                if h == 0:
                    for q in range(2, NQ):
                        nc.sync.dma_start(
                            vt4[q][:], vv[:, q * 2048:(q + 1) * 2048])
                    for j in range(4):
                        nc.sync.dma_start(
                            wopt[j][:],
                            wop[:, j * 4 * OUT_C:(j + 1) * 4 * OUT_C])

                # value phase (head 1 in token-halves so its first o2
                # AllGather launches at the midpoint)
                o_sb = [att.tile([128, M], F16, name=f"o_{h}_{c}")
                        for c in range(4)]
                o2s = att.tile([128, M], F16, name=f"o2s{h}")
                halves = [(0, M)]
                with (
                    tc.tile_pool(name=f"psO{h}", bufs=1, space="PSUM") as psO,
                    tc.tile_pool(name=f"psB{h}", bufs=1, space="PSUM") as psB,
                ):
                    zsum = psB.tile([1, M], F32, name="zsum")
                    zb = psB.tile([128, M], F32, name="zb")
                    acc5h = [psB.tile([128, hi - lo], F32, name=f"acc5_{hv}")
                             for hv, (lo, hi) in enumerate(halves)]
                    o_ps = [psO.tile([128, M], F32, name=f"op{c}")
                            for c in range(4)]
                    for hv, (lo, hi) in enumerate(halves):
                        w = hi - lo
                        for sc in range(NSC):
                            q, r = divmod(sc, 4)
                            psl = pt[:, sc * M + lo:sc * M + hi]
                            for c in range(4):
                                nc.tensor.matmul(
                                    o_ps[c][:, 0:w],
                                    vt4[q][:, r * 512 + c * 128:
                                           r * 512 + (c + 1) * 128],
                                    psl, start=(sc == 0),
                                    stop=(sc == NSC - 1),
                                    skip_group_check=True)
                            if hv == 0 and sc == 2:
                                nc.tensor.matmul(zsum[:], ones_col[:],
                                                 zf16[:], start=True,
                                                 stop=True,
                                                 skip_group_check=True)
                                nc.vector.tensor_copy(z_sb[:], zsum[:])
                                nc.vector.reciprocal(rz[:], z_sb[:])
                            if hv == 0 and sc == 8:
                                nc.tensor.matmul(zb[:], ones_row[:], rz[:],
                                                 start=True, stop=True,
                                                 skip_group_check=True)
                                nc.vector.tensor_copy(zb_sb[h][:], zb[:])
                        for c in range(4):
                            nc.vector.tensor_copy(o_sb[c][:, lo:hi],
                                                  o_ps[c][:, 0:w])
                        for kc in range(4):
                            nc.tensor.matmul(
                                acc5h[hv][:], wo1t[h][kc][:],
                                o_sb[kc][:, lo:hi],
                                start=(kc == 0), stop=(kc == 3),
                                skip_group_check=True)
                        nc.vector.tensor_mul(o2s[:, lo:hi], acc5h[hv][:],
                                             zb_sb[h][:, lo:hi])
                        nc.sync.dma_start(o2_loc[h][hv][:], o2s[:, lo:hi])
                        nc.gpsimd.collective_compute(
                            "AllGather", mybir.AluOpType.bypass,
                            replica_groups=rg,
                            ins=[o2_loc[h][hv].opt()],
                            outs=[o2_all[h][hv].opt()])
                att_cm.__exit__(None, None, None)

            # ---------------- O-projection --------------------------------
            def wslice(j, p):
                # stationary [128,128] for contraction block j, out chunk p
                return wopt[j // 4][:, (j % 4) * OUT_C + p * 128:
                                   (j % 4) * OUT_C + (p + 1) * 128]

            with (
                tc.tile_pool(name="s6", bufs=1) as s6,
                tc.tile_pool(name="ps6", bufs=1, space="PSUM") as ps6,
                tc.tile_pool(name="s6o", bufs=3) as s6o,
            ):
                acc6 = [ps6.tile([128, M], F32, name=f"acc6_{p}")
                        for p in range(7)]
                # part A: head-0 blocks only (hides head-1's AllGather)
                for k in range(8):
                    for p in range(7):
                        nc.tensor.matmul(
                            acc6[p][:], wslice(k, p), o2t[0][k][:],
                            start=(k == 0), stop=False,
                            skip_group_check=True)
                # part B: head-1 blocks (k-major so the first read
                # unblocks compute while the rest stream in)
                o2t1 = [None] * 8
                for k in range(8):
                    ok = s6.tile([128, M], F16, name=f"o2a1_{k}")
                    nc.sync.dma_start(
                        ok[:], o2_all[1][0][k * 128:(k + 1) * 128, :])
                    o2t1[k] = ok
                for k in range(7):
                    for p in range(7):
                        nc.tensor.matmul(
                            acc6[p][:], wslice(8 + k, p), o2t1[k][:],
                            start=False, stop=False,
                            skip_group_check=True)
                for p in range(7):
                    nc.tensor.matmul(
                        acc6[p][:], wslice(15, p), o2t1[7][:],
                        start=False, stop=True, skip_group_check=True)
                    ob = s6o.tile([128, M], F16, name="outb", tag="outb")
                    if p % 2 == 0:
                        nc.vector.tensor_copy(ob[:], acc6[p][:])
                    else:
                        nc.scalar.activation(
                            ob[:], acc6[p][:],
                            mybir.ActivationFunctionType.Copy)
                    nc.sync.dma_start(outT[p * 128:(p + 1) * 128, :], ob[:])

            glob_cm.__exit__(None, None, None)

    nc.compile()
    return nc


def _pack(a, nblk):
    """[nblk*128, W] -> [128, nblk*W] with block-major free layout."""
    w = a.shape[1]
    return np.ascontiguousarray(
        a.reshape(nblk, 128, w).transpose(1, 0, 2).reshape(128, nblk * w))


def prep_inputs(x, W_cqkv, W_uq, W_qk, kv_cache, W_o1, W_oproj, indices):
    x = np.asarray(x, np.float32)
    W_cqkv = np.asarray(W_cqkv, np.float32)
    W_uq = np.asarray(W_uq, np.float32)
    W_qk = np.asarray(W_qk, np.float32)
    kv_cache = np.asarray(kv_cache, np.float32)
    W_o1 = np.asarray(W_o1, np.float32)
    W_oproj = np.asarray(W_oproj, np.float32)
    indices = np.asarray(indices)

    xTp = _pack(np.ascontiguousarray(x.T), KH).astype(NP16)
    wq_full = W_cqkv[:, D_KV_C:D_KV_C + 1536]

    kvT = np.ascontiguousarray(kv_cache.T)  # [576, 4096]
    # kvA[p, sc*512 + j*128 + c] = kv dim (j*128+p) at position (sc*128+c)
    kvAf = np.ascontiguousarray(
        kvT[:512].reshape(4, 128, NSC, 128).transpose(1, 2, 0, 3)
        .reshape(128, NSC * 512)).astype(NP16)
    kvBf = np.ascontiguousarray(kvT[512:]).astype(NP16)
    vvP = _pack(np.ascontiguousarray(kv_cache[:, :D_KV_C]),
                NSC).astype(NP16)

    cm = np.bincount(
        (np.arange(M, dtype=np.int64)[:, None] * S_KV + indices).ravel(),
        minlength=M * S_KV).reshape(M, S_KV)
    cntP = _pack(np.ascontiguousarray(cm.T.astype(np.float32)),
                 NSC).astype(NP16)

    shf_np = np.zeros((128, 64), NP16)
    shf_np[np.arange(64) + 64, np.arange(64)] = 1.0

    in_maps = []
    for i in range(N_CORES):
        h0 = i * H_LOC
        c0 = i * OUT_C
        # fused q weights; cols [h0 nope | h0 pe ; h1 pe | h1 nope]
        wu = W_uq[:, h0 * 192:(h0 + H_LOC) * 192]
        cols = np.concatenate([
            wu[:, 0:128], wu[:, 128:192], wu[:, 320:384], wu[:, 192:320],
        ], axis=1)
        wqe_i = _pack((wq_full @ cols), KH).astype(NP16)
        # O-proj rows: head-0 rank blocks then head-1 rank blocks
        wop_rows = []
        for h in range(H_LOC):
            for k in range(N_CORES):
                g = k * H_LOC + h
                wop_rows.append(W_oproj[g * 128:(g + 1) * 128,
                                        c0:c0 + OUT_C])
        wop_i = _pack(np.concatenate(wop_rows, 0), 16).astype(NP16)
        in_maps.append({
            "xT": xTp,
            "wqe": wqe_i,
            "shf": shf_np,
            "wqk": W_qk[h0:h0 + H_LOC].astype(NP16),
            "kvA": kvAf,
            "kvB": kvBf,
            "vv": vvP,
            "cnt": cntP,
            "wo1": W_o1[h0:h0 + H_LOC].astype(NP16),
            "wop": wop_i,
        })
    return in_maps


_prog_cache = {}


def kernel(x, W_cqkv, W_uq, W_qk, kv_cache, W_o1, W_oproj, indices):
    if "nc" not in _prog_cache:
        _prog_cache["nc"] = build_program()
    nc = _prog_cache["nc"]
    in_maps = prep_inputs(x, W_cqkv, W_uq, W_qk, kv_cache, W_o1, W_oproj,
                          indices)
    trace = bool(int(os.environ.get("KERNEL_TRACE", "0")))
    res = run_bass_kernel_spmd(nc, in_maps, list(range(N_CORES)),
                               trace=trace)
    _prog_cache["last_result"] = res
    out = np.empty((M, HID), np.float32)
    for i in range(N_CORES):
        out[:, i * OUT_C:(i + 1) * OUT_C] = res.results[i]["outT"].T
    return out
